# revision 2
# baseline (speedup 1.0000x reference)
"""MoE top-2 routing kernel for Trainium2 (8 NeuronCores, expert-parallel).

Host: gating softmax + top-2 (float64 numpy), per-expert token gather,
weight re-layout + bf16 cast. Device (per core, SPMD): one expert's MLP
   h = relu(x @ W1 + b1); y = h @ W2
over that expert's routed tokens, bf16 matmuls with fp32 PSUM accumulation.
Host: combine y * gate (+ b2 folded in) via scatter-add.

Orientation on device: tokens live in the matmul free dim, feature dims in
the partition dim, so both layers chain without transposes and b1 is a
per-partition activation bias. Output is [D, cap] per core, transposed back
on host.
"""

import numpy as np
import ml_dtypes

import concourse.bass as bass
from concourse import mybir
from concourse.bass_utils import run_bass_kernel_spmd

D = 1024
HID = 4096
E = 8
TOP_K = 2
KD = D // 128     # 8  k-blocks for layer 1
MH = HID // 128   # 32 m-blocks for layer 1 / k-blocks for layer 2
MD = D // 128     # 8  m-blocks for layer 2
TC = 512          # token chunk (matmul free dim / PSUM bank)

BF16 = ml_dtypes.bfloat16


def _build_program(cap: int):
    """Raw-bass SPMD program for one expert MLP over `cap` routed tokens."""
    n_chunks = (cap + TC - 1) // TC
    chunks = []  # (chunk_idx, t0, tc)
    for c in range(n_chunks):
        t0 = c * TC
        chunks.append((c, t0, min(TC, cap - t0)))

    nc = bass.Bass()

    xTd = nc.dram_tensor("xT", [KD, 128, cap], mybir.dt.bfloat16, kind="ExternalInput")
    w1d = nc.dram_tensor("w1", [MH, 128, KD * 128], mybir.dt.bfloat16, kind="ExternalInput")
    b1d = nc.dram_tensor("b1t", [128, MH], mybir.dt.float32, kind="ExternalInput")
    w2d = nc.dram_tensor("w2", [MD, 128, MH * 128], mybir.dt.bfloat16, kind="ExternalInput")
    outd = nc.dram_tensor("outT", [MD, 128, cap], mybir.dt.float32, kind="ExternalOutput")

    with (
        nc.sbuf_tensor("w1_sb", [128, MH * KD * 128], mybir.dt.bfloat16) as w1_sb,
        nc.sbuf_tensor("w2_sb", [128, MD * MH * 128], mybir.dt.bfloat16) as w2_sb,
        nc.sbuf_tensor("x_sb", [128, 2 * KD * TC], mybir.dt.bfloat16) as x_sb,
        nc.sbuf_tensor("h_sb", [128, MH * TC], mybir.dt.bfloat16) as h_sb,
        nc.sbuf_tensor("o_sb", [128, 2 * TC], mybir.dt.float32) as o_sb,
        nc.sbuf_tensor("b1_sb", [128, MH], mybir.dt.float32) as b1_sb,
        nc.psum_tensor("pt1a", [128, TC], mybir.dt.float32) as pt1a,
        nc.psum_tensor("pt1b", [128, TC], mybir.dt.float32) as pt1b,
        nc.psum_tensor("pt2a", [128, TC], mybir.dt.float32) as pt2a,
        nc.psum_tensor("pt2b", [128, TC], mybir.dt.float32) as pt2b,
        nc.semaphore("dma_misc") as dma_misc,   # b1 load
        nc.semaphore("dma_x") as dma_x,         # +16 per x-chunk-k DMA
        nc.semaphore("dma_w1") as dma_w1,       # +16 per w1 m-block DMA
        nc.semaphore("dma_w2") as dma_w2,       # +16 per w2 mo-block DMA
        nc.semaphore("dma_out") as dma_out,     # +16 per out DMA
        nc.semaphore("pe1_sem") as pe1_sem,     # +1 per finished L1 m-group
        nc.semaphore("pe2_sem") as pe2_sem,     # +1 per finished L2 mo-group
        nc.semaphore("act1_sem") as act1_sem,   # +1 per L1 psum evict (relu)
        nc.semaphore("dve_sem") as dve_sem,     # +1 per L2 psum evict (copy)
        nc.Block() as block,
    ):
        pt1 = [pt1a, pt1b]
        pt2 = [pt2a, pt2b]

        @block.sync
        def _(sync):
            sync.dma_start(out=b1_sb[:], in_=b1d[:]).then_inc(dma_misc, 16)
            # x chunk 0 and 1, w1, w2 up-front; later x chunks gated on PE
            for c, t0, tc in chunks[:2]:
                xoff = (c % 2) * KD * TC
                for k in range(KD):
                    sync.dma_start(
                        out=x_sb[:, xoff + k * TC: xoff + k * TC + tc],
                        in_=xTd[k, :, t0: t0 + tc],
                    ).then_inc(dma_x, 16)
            for m in range(MH):
                sync.dma_start(
                    out=w1_sb[:, m * KD * 128: (m + 1) * KD * 128],
                    in_=w1d[m],
                ).then_inc(dma_w1, 16)
            for mo in range(MD):
                sync.dma_start(
                    out=w2_sb[:, mo * MH * 128: (mo + 1) * MH * 128],
                    in_=w2d[mo],
                ).then_inc(dma_w2, 16)
            for c, t0, tc in chunks:
                # prefetch x chunk c+2 once its buffer is free (L1 of chunk c done)
                if c + 2 < n_chunks:
                    c2, t02, tc2 = chunks[c + 2]
                    sync.wait_ge(pe1_sem, MH * (c + 1))
                    xoff = (c2 % 2) * KD * TC
                    for k in range(KD):
                        sync.dma_start(
                            out=x_sb[:, xoff + k * TC: xoff + k * TC + tc2],
                            in_=xTd[k, :, t02: t02 + tc2],
                        ).then_inc(dma_x, 16)
                # store outputs of chunk c as they are evicted
                for mo in range(MD):
                    g = c * MD + mo
                    sync.wait_ge(dve_sem, g + 1)
                    sync.dma_start(
                        out=outd[mo, :, t0: t0 + tc],
                        in_=o_sb[:, (mo % 2) * TC: (mo % 2) * TC + tc],
                    ).then_inc(dma_out, 16)

        @block.tensor
        def _(tensor):
            for c, t0, tc in chunks:
                xoff = (c % 2) * KD * TC
                tensor.wait_ge(dma_x, 16 * KD * (c + 1))
                # layer 1: h[m*128+p, t] = sum_d W1[d, m*128+p] x[d, t]
                for m in range(MH):
                    if c == 0:
                        tensor.wait_ge(dma_w1, 16 * (m + 1))
                    g1 = c * MH + m
                    if g1 >= 2:
                        tensor.wait_ge(act1_sem, g1 - 1)  # psum bank m%2 evicted
                    ps = pt1[m % 2]
                    for k in range(KD):
                        mm = tensor.matmul(
                            ps[:, :tc],
                            w1_sb[:, (m * KD + k) * 128: (m * KD + k + 1) * 128],
                            x_sb[:, xoff + k * TC: xoff + k * TC + tc],
                            start=(k == 0),
                            stop=(k == KD - 1),
                        )
                    mm.then_inc(pe1_sem, 1)
                # layer 2: y[mo*128+p, t] = sum_hid W2[hid, mo*128+p] h[hid, t]
                tensor.wait_ge(act1_sem, MH * (c + 1))  # all h of this chunk ready
                for mo in range(MD):
                    if c == 0:
                        tensor.wait_ge(dma_w2, 16 * (mo + 1))
                    g2 = c * MD + mo
                    if g2 >= 2:
                        tensor.wait_ge(dve_sem, g2 - 1)  # psum bank mo%2 evicted
                    ps = pt2[mo % 2]
                    for k in range(MH):
                        mm = tensor.matmul(
                            ps[:, :tc],
                            w2_sb[:, (mo * MH + k) * 128: (mo * MH + k + 1) * 128],
                            h_sb[:, k * TC: k * TC + tc],
                            start=(k == 0),
                            stop=(k == MH - 1),
                        )
                    mm.then_inc(pe2_sem, 1)

        @block.scalar
        def _(scalar):
            scalar.wait_ge(dma_misc, 16)
            for c, t0, tc in chunks:
                for m in range(MH):
                    if m == 0 and c > 0:
                        # h_sb reused: wait until L2 of chunk c-1 consumed it
                        scalar.wait_ge(pe2_sem, MD * c)
                    scalar.wait_ge(pe1_sem, c * MH + m + 1)
                    scalar.activation(
                        h_sb[:, m * TC: m * TC + tc],
                        pt1[m % 2][:, :tc],
                        mybir.ActivationFunctionType.Relu,
                        bias=b1_sb[:, m: m + 1],
                    ).then_inc(act1_sem, 1)

        @block.vector
        def _(vector):
            for c, t0, tc in chunks:
                for mo in range(MD):
                    g = c * MD + mo
                    if g >= 2:
                        vector.wait_ge(dma_out, 16 * (g - 1))  # o_sb slot stored
                    vector.wait_ge(pe2_sem, g + 1)
                    vector.tensor_copy(
                        o_sb[:, (mo % 2) * TC: (mo % 2) * TC + tc],
                        pt2[mo % 2][:, :tc],
                    ).then_inc(dve_sem, 1)

    return nc


def kernel(x, Wg, bg, W1, b1, W2, b2):
    x = np.asarray(x)
    xt = x.reshape(-1, D).astype(np.float32, copy=False)
    N = xt.shape[0]

    # --- gating on host, float64 to keep top-k selection faithful to the
    # fp32 reference (true gate margins >> fp32 rounding noise)
    logits = xt.astype(np.float64) @ np.asarray(Wg).astype(np.float64)
    logits += np.asarray(bg).astype(np.float64)
    logits -= logits.max(axis=-1, keepdims=True)
    gates = np.exp(logits)
    gates /= gates.sum(axis=-1, keepdims=True)
    order = np.argsort(-gates, axis=-1)[:, :TOP_K]            # [N, K]
    topw = np.take_along_axis(gates, order, axis=-1)          # [N, K]

    # --- per-expert token lists
    idx_e = []
    gate_e = []
    for e in range(E):
        sel = (order == e)
        rows = np.nonzero(sel.any(axis=1))[0]
        w = (topw * sel).sum(axis=1)[rows]
        idx_e.append(rows)
        gate_e.append(w.astype(np.float32))
    counts = np.array([len(r) for r in idx_e])
    cap = max(512, int(-(-counts.max() // 128) * 128))

    # --- build per-core inputs
    W1 = np.asarray(W1, dtype=np.float32)
    W2 = np.asarray(W2, dtype=np.float32)
    b1 = np.asarray(b1, dtype=np.float32)
    b2 = np.asarray(b2, dtype=np.float32)
    in_maps = []
    for e in range(E):
        xe = np.zeros((cap, D), dtype=np.float32)
        xe[: counts[e]] = xt[idx_e[e]]
        xT = np.ascontiguousarray(xe.T).reshape(KD, 128, cap).astype(BF16)
        # w1[m, p, k*128+j] = W1[e, k*128+p, m*128+j]
        w1r = np.ascontiguousarray(
            W1[e].reshape(KD, 128, MH, 128).transpose(2, 1, 0, 3).reshape(MH, 128, KD * 128)
        ).astype(BF16)
        # w2[mo, p, k*128+j] = W2[e, k*128+p, mo*128+j]
        w2r = np.ascontiguousarray(
            W2[e].reshape(MH, 128, MD, 128).transpose(2, 1, 0, 3).reshape(MD, 128, MH * 128)
        ).astype(BF16)
        b1r = np.ascontiguousarray(b1[e].reshape(MH, 128).T)
        in_maps.append({"xT": xT, "w1": w1r, "b1t": b1r, "w2": w2r})

    nc = _build_program(cap)
    res = run_bass_kernel_spmd(nc, in_maps, core_ids=list(range(E)))
    global _last_results
    _last_results = res

    # --- combine on host: out[n] = sum_e gate[n,e] * (mlp_e(x[n]) + b2[e])
    out = np.zeros((N, D), dtype=np.float32)
    for e in range(E):
        ye = res.results[e]["outT"].reshape(D, cap).T  # [cap, D]
        out[idx_e[e]] += gate_e[e][:, None] * (ye[: counts[e]] + b2[e])
    return out.reshape(x.shape).astype(np.float32)
